# revision 49
# baseline (speedup 1.0000x reference)
"""Trainium2 Bass kernel for the ConceptNet-style module.

Computes, distributed data-parallel over 8 NeuronCores:
    y_pred = train_embedding @ concept @ inv(concept.T@concept) @ (concept.T@W_h) + b_h
    L1     = sum(score_norm);  L2 = sum(G) - trace(G),  G = score_norm.T@score_norm
where score_norm is the column-normalized |mean_cs(clusters) @ concept_normalized|
(the concept column normalization cancels exactly inside the score normalization,
so the kernel works with |cluster_sum @ concept| directly — uniform positive
column scalings cancel in both L1 and L2).

Sharding (hardcoded): train_embedding split 8 x [1024, 768] along batch;
clusters padded 500->504 and split 8 x [63, 100, 768] along the cluster dim
(each shard flattened to [6300, 768], zero-padded to [6400, 768]); concept,
W_h, b_h replicated. Each core returns its y_pred shard plus tiny partial
reductions (G0p = S0.T@S0 [64,64] and column sums of S0 [64]); the host sums
partials across cores and finishes the two scalar losses (the "all-reduce the
scalar sparsity losses" step of the data-parallel recipe).

inv(gram) is computed on-device with Newton-Schulz iterations (X0 = I/||G||_F,
X <- 2X - X G X), which converges to fp32 accuracy in <=10 iterations for this
well-conditioned 64x64 Gram matrix.
"""

import os
import sys

import ml_dtypes
import numpy as np

for _p in ("/opt/trn_rl_repo", os.path.expanduser("~/.axon_site/_ro/trn_rl_repo")):
    if os.path.isdir(_p) and _p not in sys.path:
        sys.path.insert(0, _p)

import concourse.bass as bass
import concourse.mybir as mybir
from concourse import bacc
from concourse.bass_utils import run_bass_kernel_spmd
from concourse.masks import make_identity
from concourse.tile import TileContext

EPS = 1e-12
N_CORES = 8
BS, D, NCPT, NCL, CS, NY = 8192, 768, 64, 500, 100, 1000
BS_SH = BS // N_CORES            # 1024 batch rows per core
NCL_SH = 63                      # ceil(500/8) clusters per core (504 padded)
ROWS_SH = NCL_SH * CS            # 6300
NT_CL = 50                       # ceil(6300/128); last tile has 28 valid rows
LAST_K = ROWS_SH - (NT_CL - 1) * 128  # 28
NT_CL_A = 25                     # tiles in the first CM accumulation phase
NT_TE = BS_SH // 128             # 8
DC = D // 128                    # 6 contraction chunks of 128
NYH = NY // 2                    # 500 (psum-bank-sized y/n chunks)
NS_ITERS = 9

DT = mybir.dt.float32
BF = mybir.dt.bfloat16
FP32 = np.float32

# Toggled by test harness; kernel() stores profiling results here.
TRACE = False
LAST_EXEC_NS = None
LAST_TRACE_PATH = None
LAST_RESULTS = None

_cached_nc = None


def _cluster_map() -> np.ndarray:
    """Block-ones row->cluster map, pre-laid-out as [128, NT_CL*63] so the
    SBUF load is one DMA with contiguous per-partition rows (map[p, t*63+m]
    = 1 iff global row t*128+p belongs to cluster m).

    bf16: entries are 0/1 (exact), and the cluster-sum path is scale-invariant
    in the final losses, so low precision here only rounds the summands."""
    m = np.zeros((NT_CL * 128, NCL_SH), dtype=ml_dtypes.bfloat16)
    g = np.arange(ROWS_SH)
    m[g, g // CS] = 1.0
    return np.ascontiguousarray(
        m.reshape(NT_CL, 128, NCL_SH).transpose(1, 0, 2).reshape(128, NT_CL * NCL_SH))


def _build_program() -> bass.Bass:
    # Bacc (not raw Bass): its compile() legalizes matmul sync waits
    # (move_matmul_waits_to_ldweights + nop/event fusion) — the trn2 LDWEIGHTS
    # ISA struct carries no wait fields, so raw-Bass TileContext programs with
    # cross-engine matmul deps fail walrus codegen ("Too many sync wait").
    nc = bacc.Bacc()

    te = nc.declare_dram_parameter("te", [BS_SH, D], DT, isOutput=False)
    cl = nc.declare_dram_parameter("cl", [ROWS_SH, D], DT, isOutput=False)
    # concept pre-transposed on host to [128, DC*64] (partition-contiguous)
    co_p = nc.declare_dram_parameter("co_p", [128, DC * NCPT], DT, isOutput=False)
    wh = nc.declare_dram_parameter("wh", [D, NY], DT, isOutput=False)
    bh = nc.declare_dram_parameter("bh", [NY], DT, isOutput=False)
    y_out = nc.declare_dram_parameter("Y", [BS_SH, NY], DT, isOutput=True)
    g0_out = nc.declare_dram_parameter("G0p", [NCPT, NCPT], DT, isOutput=True)
    cs_out = nc.declare_dram_parameter("cs1", [1, NCPT], DT, isOutput=True)

    cmap = nc.inline_tensor(_cluster_map(), name="cmap")

    with TileContext(nc) as tc:
        with (
            tc.tile_pool(name="const", bufs=1) as const,
            tc.tile_pool(name="small", bufs=2) as small,
            tc.tile_pool(name="nsx", bufs=3) as nsx,
            tc.tile_pool(name="clp", bufs=18) as clp,
            tc.tile_pool(name="clbp", bufs=9) as clbp,
            tc.tile_pool(name="tep", bufs=4) as tep,
            tc.tile_pool(name="tetp", bufs=1) as tetp,
            tc.tile_pool(name="ap", bufs=2) as apool,
            tc.tile_pool(name="yp", bufs=3) as ypool,
            tc.tile_pool(name="ps_small", bufs=2, space="PSUM") as ps_small,
            tc.tile_pool(name="ps_cm", bufs=1, space="PSUM") as ps_cm,
            tc.tile_pool(name="ps_wide", bufs=2, space="PSUM") as ps_wide,
        ):
            # ---- constants / replicated operands ----
            ident = const.tile([128, 128], DT)
            make_identity(nc, ident)
            ones63 = const.tile([63, 1], DT)
            nc.vector.memset(ones63[:], 1.0)
            ones64 = const.tile([NCPT, 1], DT)
            nc.vector.memset(ones64[:], 1.0)
            ones1 = const.tile([1, NCPT], DT)
            nc.vector.memset(ones1[:], 1.0)

            co_sb = const.tile([128, DC * NCPT], DT)
            nc.sync.dma_start(co_sb[:], co_p[:])

            map_sb = const.tile([128, NT_CL * NCL_SH], BF)
            _mh = NT_CL_A * NCL_SH
            nc.sync.dma_start(map_sb[:, :_mh], cmap[:, :_mh])
            nc.sync.dma_start(map_sb[:, _mh:], cmap[:, _mh:])

            def co_c(c):
                return co_sb[:, c * NCPT:(c + 1) * NCPT]

            # ---- gram = concept.T @ concept ----
            gram_ps = ps_small.tile([NCPT, NCPT], DT, tag="ps_small")
            for c in range(DC):
                nc.tensor.matmul(gram_ps[:], co_c(c), co_c(c),
                                 start=(c == 0), stop=(c == DC - 1))
            G_sb = small.tile([NCPT, NCPT], DT, tag="gsb")
            nc.vector.tensor_copy(G_sb[:], gram_ps[:])

            # ---- clusters: start streaming early; CM accumulates in PSUM ----
            # one PSUM tile per 384-wide half: a matmul output must stay
            # within a single 512-f32 PSUM bank. Two accumulation phases
            # (tiles 0..24, 25..49) so most of the transpose/T finalize chain
            # hides mid-kernel instead of trailing the last cluster DMA.
            def new_cm_halves(ph):
                return [ps_cm.tile([NCL_SH, 384], DT, tag=f"cm{h}",
                                   name=f"cm_{ph}_{h}") for h in range(2)]

            cm_halves = new_cm_halves("a")

            def cluster_tile(t, t_first, t_last):
                k = LAST_K if t == NT_CL - 1 else 128
                cl_t = clp.tile([128, D], DT, tag="cl")
                nc.sync.dma_start(cl_t[:k, :], cl[t * 128:t * 128 + k, :])
                cl_b = clbp.tile([128, D], BF, tag="clb", name="cl_b")
                nc.vector.tensor_copy(cl_b[:k, :], cl_t[:k, :])
                for h in range(2):
                    nc.tensor.matmul(cm_halves[h][:],
                                     map_sb[:k, t * NCL_SH:(t + 1) * NCL_SH],
                                     cl_b[:k, h * 384:(h + 1) * 384],
                                     start=(t == t_first), stop=(t == t_last))

            for t in range(4):
                cluster_tile(t, 0, NT_CL_A - 1)

            # W_h loads deferred past the first cluster tiles so the cluster
            # stream (the DMA-bound critical path) starts immediately
            wh_sb = const.tile([128, DC, NY], DT)
            wh_re = wh.rearrange("(c p) n -> p c n", p=128)
            for c in range(DC):
                nc.sync.dma_start(wh_sb[:, c, :], wh_re[:, c, :])

            # prefetch all train-embedding row tiles; compute interleaves later
            te_tiles = []
            for rr in range(NT_TE):
                te_t = tep.tile([128, D], DT, tag="te", name=f"te_t{rr}")
                nc.sync.dma_start(te_t[:], te[rr * 128:(rr + 1) * 128, :])
                te_tiles.append(te_t)

            # ---- Newton-Schulz inverse of gram ----
            # t0 = 1/||G||_F broadcast to a [64,1] per-partition scalar column
            h2 = small.tile([NCPT, NCPT], DT, tag="h2")
            nc.scalar.square(h2[:], G_sb[:])
            hrow = small.tile([NCPT, 1], DT, tag="hrow")
            nc.vector.tensor_reduce(hrow[:], h2[:], axis=mybir.AxisListType.X,
                                    op=mybir.AluOpType.add)
            hs_ps = ps_small.tile([1, 1], DT, tag="ps_small")
            nc.tensor.matmul(hs_ps[:], ones64[:], hrow[:], start=True, stop=True)
            fro = small.tile([1, 1], DT, tag="fro")
            nc.scalar.sqrt(fro[:], hs_ps[:])
            tinv = small.tile([1, 1], DT, tag="tinv")
            nc.vector.reciprocal(tinv[:], fro[:])
            tc_ps = ps_small.tile([NCPT, 1], DT, tag="ps_small")
            nc.tensor.matmul(tc_ps[:], ones1[:], tinv[:], start=True, stop=True)
            tcol = small.tile([NCPT, 1], DT, tag="tcol")
            nc.vector.tensor_copy(tcol[:], tc_ps[:])

            X = nsx.tile([NCPT, NCPT], DT, tag="X")
            nc.vector.tensor_copy(X[:], ident[:NCPT, :NCPT])
            nc.vector.tensor_scalar_mul(X[:], X[:], tcol[:])
            for i in range(NS_ITERS):
                e_ps = ps_small.tile([NCPT, NCPT], DT, tag="ps_small")
                nc.tensor.matmul(e_ps[:], G_sb[:], X[:], start=True, stop=True)
                e_sb = small.tile([NCPT, NCPT], DT, tag="esb")
                nc.scalar.copy(e_sb[:], e_ps[:])
                xe_ps = ps_small.tile([NCPT, NCPT], DT, tag="ps_small")
                nc.tensor.matmul(xe_ps[:], X[:], e_sb[:], start=True, stop=True)
                xn = nsx.tile([NCPT, NCPT], DT, tag="X")
                nc.vector.scalar_tensor_tensor(
                    out=xn[:], in0=X[:], scalar=2.0, in1=xe_ps[:],
                    op0=mybir.AluOpType.mult, op1=mybir.AluOpType.subtract)
                X = xn

            # ---- B' = [inv(gram) @ (concept.T @ W_h); b_h]  (65 x 1000) ----
            cw_sb = small.tile([NCPT, NY], DT, tag="cwsb")
            for hh in range(2):
                cw_ps = ps_wide.tile([NCPT, NYH], DT, tag="ps_wide")
                for c in range(DC):
                    nc.tensor.matmul(cw_ps[:],
                                     co_c(c), wh_sb[:, c, hh * NYH:(hh + 1) * NYH],
                                     start=(c == 0), stop=(c == DC - 1))
                nc.vector.tensor_copy(cw_sb[:, hh * NYH:(hh + 1) * NYH], cw_ps[:])
            bp_sb = const.tile([65, NY], DT)
            for hh in range(2):
                b_ps = ps_wide.tile([NCPT, NYH], DT, tag="ps_wide")
                nc.tensor.matmul(b_ps[:], X[:], cw_sb[:, hh * NYH:(hh + 1) * NYH],
                                 start=True, stop=True)
                nc.vector.tensor_copy(bp_sb[:NCPT, hh * NYH:(hh + 1) * NYH], b_ps[:])
            nc.gpsimd.dma_start(bp_sb[NCPT:NCPT + 1, :], bh[None, :])

            # ---- interleave: clusters stream + y_pred path in groups of 4
            # row tiles, so the A^T matmuls run at N=512 (fewer, wider
            # instructions: per-matmul dispatch overhead is ~200ns)
            def te_group(g):
                tet4 = tetp.tile([128, DC, 512], DT, tag="tet", name=f"tet{g}")
                for j in range(4):
                    te_t = te_tiles[4 * g + j]
                    for c in range(DC):
                        tp_ps = ps_small.tile([128, 128], DT, tag="ps_small")
                        nc.tensor.transpose(tp_ps[:], te_t[:, c * 128:(c + 1) * 128],
                                            ident[:])
                        # ACT copy: keep DVE free for the cluster-stream casts
                        nc.scalar.copy(tet4[:, c, j * 128:(j + 1) * 128],
                                       tp_ps[:])
                at_ps = ps_wide.tile([NCPT, 512], DT, tag="ps_wide")
                for c in range(DC):
                    nc.tensor.matmul(at_ps[:], co_c(c), tet4[:, c, :],
                                     start=(c == 0), stop=(c == DC - 1))
                a_sb = apool.tile([65, 512], DT, tag="a")
                nc.vector.tensor_copy(a_sb[:NCPT, :], at_ps[:])
                nc.vector.memset(a_sb[NCPT:NCPT + 1, :], 1.0)
                for j in range(4):
                    r = 4 * g + j
                    y_sb = ypool.tile([128, NY], DT, tag="y")
                    for hh in range(2):
                        y_ps = ps_wide.tile([128, NYH], DT, tag="ps_wide")
                        nc.tensor.matmul(y_ps[:], a_sb[:, j * 128:(j + 1) * 128],
                                         bp_sb[:, hh * NYH:(hh + 1) * NYH],
                                         start=True, stop=True)
                        nc.vector.tensor_copy(y_sb[:, hh * NYH:(hh + 1) * NYH],
                                              y_ps[:])
                    nc.gpsimd.dma_start(y_out[r * 128:(r + 1) * 128, :], y_sb[:])

            # CM phase finalize: copy psum halves out, transpose to CM^T,
            # T += CM^T @ concept accumulated in SBUF t_acc across phases
            t_acc = small.tile([NCL_SH, NCPT], DT, tag="tacc")

            def cm_finalize(ph, first_phase):
                cm_sb = small.tile([NCL_SH, D], DT, tag="cmsb",
                                   name=f"cm_sb_{ph}")
                for h in range(2):
                    nc.vector.tensor_copy(cm_sb[:, h * 384:(h + 1) * 384],
                                          cm_halves[h][:])
                cmt = small.tile([128, DC, NCL_SH], DT, tag="cmt",
                                 name=f"cmt_{ph}")
                for c in range(DC):
                    tp_ps = ps_small.tile([128, 128], DT, tag="ps_small")
                    nc.tensor.transpose(tp_ps[:, :NCL_SH],
                                        cm_sb[:, c * 128:(c + 1) * 128],
                                        ident[:NCL_SH, :NCL_SH])
                    nc.vector.tensor_copy(cmt[:, c, :], tp_ps[:, :NCL_SH])
                t_ps = ps_small.tile([NCL_SH, NCPT], DT, tag="ps_small")
                for c in range(DC):
                    nc.tensor.matmul(t_ps[:], cmt[:, c, :], co_c(c),
                                     start=(c == 0), stop=(c == DC - 1))
                if first_phase:
                    nc.vector.tensor_copy(t_acc[:], t_ps[:])
                else:
                    nc.vector.tensor_add(out=t_acc[:], in0=t_acc[:], in1=t_ps[:])

            # front-load te compute (B' is ready ~20us in) so the kernel tail
            # is pure cluster streaming + the short loss finalize
            for t in range(4, NT_CL):
                if t == NT_CL_A:
                    cm_finalize("a", True)
                    cm_halves = new_cm_halves("b")
                cluster_tile(t, NT_CL_A if t >= NT_CL_A else 0,
                             (NT_CL_A - 1) if t < NT_CL_A else (NT_CL - 1))
                if t == 12:
                    te_group(0)
                if t == 30:
                    te_group(1)

            # ---- loss partials tail: second CM phase, S0, G0p, colsums ----
            cm_finalize("b", False)
            s0 = small.tile([NCL_SH, NCPT], DT, tag="s0")
            nc.scalar.activation(s0[:], t_acc[:], mybir.ActivationFunctionType.Abs)
            g0_ps = ps_small.tile([NCPT, NCPT], DT, tag="ps_small")
            nc.tensor.matmul(g0_ps[:], s0[:], s0[:], start=True, stop=True)
            cs_ps = ps_small.tile([1, NCPT], DT, tag="ps_small")
            nc.tensor.matmul(cs_ps[:], ones63[:], s0[:], start=True, stop=True)
            g0_sb = small.tile([NCPT, NCPT], DT, tag="g0sb")
            nc.vector.tensor_copy(g0_sb[:], g0_ps[:])
            cs_sb = small.tile([1, NCPT], DT, tag="cssb")
            nc.vector.tensor_copy(cs_sb[:], cs_ps[:])
            nc.gpsimd.dma_start(g0_out[:], g0_sb[:])
            nc.gpsimd.dma_start(cs_out[:], cs_sb[:])

    nc.finalize()
    return nc


def _get_program() -> bass.Bass:
    global _cached_nc
    if _cached_nc is None:
        _cached_nc = _build_program()
    return _cached_nc


def kernel(train_embedding, clusters, concept, W_h, b_h):
    global LAST_EXEC_NS, LAST_TRACE_PATH, LAST_RESULTS
    te = np.ascontiguousarray(np.asarray(train_embedding, dtype=FP32))
    cl = np.asarray(clusters, dtype=FP32)
    co = np.ascontiguousarray(np.asarray(concept, dtype=FP32))
    wh = np.ascontiguousarray(np.asarray(W_h, dtype=FP32))
    bh = np.ascontiguousarray(np.asarray(b_h, dtype=FP32))
    # [768, 64] -> [128, 6*64]: partition-contiguous layout for one clean DMA
    co_p = np.ascontiguousarray(
        co.reshape(DC, 128, NCPT).transpose(1, 0, 2).reshape(128, DC * NCPT))

    # shard: batch 8 x 1024; clusters padded to 504 = 8 x 63, each shard
    # flattened to [6300, 768] (zero rows for the 4 pad clusters contribute 0)
    cl_pad = np.zeros((N_CORES * NCL_SH, CS, D), dtype=FP32)
    cl_pad[:NCL] = cl
    in_maps = []
    for i in range(N_CORES):
        sh = cl_pad[i * NCL_SH:(i + 1) * NCL_SH].reshape(ROWS_SH, D)
        in_maps.append({
            "te": np.ascontiguousarray(te[i * BS_SH:(i + 1) * BS_SH]),
            "cl": np.ascontiguousarray(sh),
            "co_p": co_p,
            "wh": wh,
            "bh": bh,
        })

    nc = _get_program()
    res = run_bass_kernel_spmd(nc, in_maps, list(range(N_CORES)), trace=TRACE)
    LAST_EXEC_NS = res.exec_time_ns
    LAST_TRACE_PATH = None
    if res.instructions_and_trace is not None:
        LAST_TRACE_PATH = res.instructions_and_trace[1]
    LAST_RESULTS = res

    y = np.concatenate([res.results[i]["Y"] for i in range(N_CORES)], axis=0)
    g0 = np.zeros((NCPT, NCPT), dtype=np.float64)
    cs1 = np.zeros(NCPT, dtype=np.float64)
    for i in range(N_CORES):
        g0 += res.results[i]["G0p"].astype(np.float64)
        cs1 += res.results[i]["cs1"][0].astype(np.float64)

    # host finalize of the two scalar losses (sum of per-core partials)
    c = np.maximum(np.sqrt(np.diag(g0)), EPS)
    l1 = (cs1 / c).sum()
    gn = g0 / np.outer(c, c)
    l2 = gn.sum() - np.trace(gn)
    return (y, np.asarray(l1, dtype=FP32), np.asarray(l2, dtype=FP32))


# revision 51
# speedup vs baseline: 1.0374x; 1.0374x over previous
"""Trainium2 Bass kernel for the ConceptNet-style module.

Computes, distributed data-parallel over 8 NeuronCores:
    y_pred = train_embedding @ concept @ inv(concept.T@concept) @ (concept.T@W_h) + b_h
    L1     = sum(score_norm);  L2 = sum(G) - trace(G),  G = score_norm.T@score_norm
where score_norm is the column-normalized |mean_cs(clusters) @ concept_normalized|
(the concept column normalization cancels exactly inside the score normalization,
so the kernel works with |cluster_sum @ concept| directly — uniform positive
column scalings cancel in both L1 and L2).

Sharding (hardcoded): train_embedding split 8 x [1024, 768] along batch;
clusters padded 500->504 and split 8 x [63, 100, 768] along the cluster dim
(each shard flattened to [6300, 768], zero-padded to [6400, 768]); concept,
W_h, b_h replicated. Each core returns its y_pred shard plus tiny partial
reductions (G0p = S0.T@S0 [64,64] and column sums of S0 [64]); the host sums
partials across cores and finishes the two scalar losses (the "all-reduce the
scalar sparsity losses" step of the data-parallel recipe).

inv(gram) is computed on-device with Newton-Schulz iterations (X0 = I/||G||_F,
X <- 2X - X G X), which converges to fp32 accuracy in <=10 iterations for this
well-conditioned 64x64 Gram matrix.
"""

import os
import sys

import ml_dtypes
import numpy as np

for _p in ("/opt/trn_rl_repo", os.path.expanduser("~/.axon_site/_ro/trn_rl_repo")):
    if os.path.isdir(_p) and _p not in sys.path:
        sys.path.insert(0, _p)

import concourse.bass as bass
import concourse.mybir as mybir
from concourse import bacc
from concourse.bass_utils import run_bass_kernel_spmd
from concourse.masks import make_identity
from concourse.tile import TileContext

EPS = 1e-12
N_CORES = 8
BS, D, NCPT, NCL, CS, NY = 8192, 768, 64, 500, 100, 1000
BS_SH = BS // N_CORES            # 1024 batch rows per core
NCL_SH = 63                      # ceil(500/8) clusters per core (504 padded)
ROWS_SH = NCL_SH * CS            # 6300
NT_CL = 50                       # ceil(6300/128); last tile has 28 valid rows
LAST_K = ROWS_SH - (NT_CL - 1) * 128  # 28
NT_CL_A = 25                     # tiles in the first CM accumulation phase
NT_TE = BS_SH // 128             # 8
DC = D // 128                    # 6 contraction chunks of 128
NYH = NY // 2                    # 500 (psum-bank-sized y/n chunks)
NS_ITERS = 9

DT = mybir.dt.float32
BF = mybir.dt.bfloat16
FP32 = np.float32

# Toggled by test harness; kernel() stores profiling results here.
TRACE = False
LAST_EXEC_NS = None
LAST_TRACE_PATH = None
LAST_RESULTS = None

_cached_nc = None


def _cluster_map() -> np.ndarray:
    """Block-ones row->cluster map, pre-laid-out as [128, NT_CL*63] so the
    SBUF load is one DMA with contiguous per-partition rows (map[p, t*63+m]
    = 1 iff global row t*128+p belongs to cluster m).

    bf16: entries are 0/1 (exact), and the cluster-sum path is scale-invariant
    in the final losses, so low precision here only rounds the summands."""
    m = np.zeros((NT_CL * 128, NCL_SH), dtype=ml_dtypes.bfloat16)
    g = np.arange(ROWS_SH)
    m[g, g // CS] = 1.0
    return np.ascontiguousarray(
        m.reshape(NT_CL, 128, NCL_SH).transpose(1, 0, 2).reshape(128, NT_CL * NCL_SH))


def _build_program() -> bass.Bass:
    # Bacc (not raw Bass): its compile() legalizes matmul sync waits
    # (move_matmul_waits_to_ldweights + nop/event fusion) — the trn2 LDWEIGHTS
    # ISA struct carries no wait fields, so raw-Bass TileContext programs with
    # cross-engine matmul deps fail walrus codegen ("Too many sync wait").
    nc = bacc.Bacc()

    te = nc.declare_dram_parameter("te", [BS_SH, D], DT, isOutput=False)
    cl = nc.declare_dram_parameter("cl", [ROWS_SH, D], DT, isOutput=False)
    # concept pre-transposed on host to [128, DC*64] (partition-contiguous)
    co_p = nc.declare_dram_parameter("co_p", [128, DC * NCPT], DT, isOutput=False)
    wh = nc.declare_dram_parameter("wh", [D, NY], DT, isOutput=False)
    bh = nc.declare_dram_parameter("bh", [NY], DT, isOutput=False)
    y_out = nc.declare_dram_parameter("Y", [BS_SH, NY], DT, isOutput=True)
    g0_out = nc.declare_dram_parameter("G0p", [NCPT, NCPT], DT, isOutput=True)
    cs_out = nc.declare_dram_parameter("cs1", [1, NCPT], DT, isOutput=True)

    cmap = nc.inline_tensor(_cluster_map(), name="cmap")

    with TileContext(nc) as tc:
        with (
            tc.tile_pool(name="const", bufs=1) as const,
            tc.tile_pool(name="small", bufs=4) as small,
            tc.tile_pool(name="nsx", bufs=3) as nsx,
            tc.tile_pool(name="clp", bufs=12) as clp,
            tc.tile_pool(name="clbp", bufs=6) as clbp,
            tc.tile_pool(name="tep", bufs=8) as tep,
            tc.tile_pool(name="tetp", bufs=2) as tetp,
            tc.tile_pool(name="ap", bufs=2) as apool,
            tc.tile_pool(name="yp", bufs=3) as ypool,
            tc.tile_pool(name="ps_small", bufs=2, space="PSUM") as ps_small,
            tc.tile_pool(name="ps_cm", bufs=1, space="PSUM") as ps_cm,
            tc.tile_pool(name="ps_wide", bufs=2, space="PSUM") as ps_wide,
        ):
            # ---- constants / replicated operands ----
            ident = const.tile([128, 128], DT)
            make_identity(nc, ident)
            ones63 = const.tile([63, 1], DT)
            nc.vector.memset(ones63[:], 1.0)
            ones64 = const.tile([NCPT, 1], DT)
            nc.vector.memset(ones64[:], 1.0)
            ones1 = const.tile([1, NCPT], DT)
            nc.vector.memset(ones1[:], 1.0)

            co_sb = const.tile([128, DC * NCPT], DT)
            nc.sync.dma_start(co_sb[:], co_p[:])

            map_sb = const.tile([128, NT_CL * NCL_SH], BF)
            _mh = NT_CL_A * NCL_SH
            nc.sync.dma_start(map_sb[:, :_mh], cmap[:, :_mh])
            nc.sync.dma_start(map_sb[:, _mh:], cmap[:, _mh:])

            def co_c(c):
                return co_sb[:, c * NCPT:(c + 1) * NCPT]

            # ---- gram = concept.T @ concept ----
            gram_ps = ps_small.tile([NCPT, NCPT], DT, tag="ps_small")
            for c in range(DC):
                nc.tensor.matmul(gram_ps[:], co_c(c), co_c(c),
                                 start=(c == 0), stop=(c == DC - 1))
            G_sb = small.tile([NCPT, NCPT], DT, tag="gsb")
            nc.vector.tensor_copy(G_sb[:], gram_ps[:])

            # ---- clusters: start streaming early; CM accumulates in PSUM ----
            # one PSUM tile per 384-wide half: a matmul output must stay
            # within a single 512-f32 PSUM bank. Two accumulation phases
            # (tiles 0..24, 25..49) so most of the transpose/T finalize chain
            # hides mid-kernel instead of trailing the last cluster DMA.
            def new_cm_halves(ph):
                return [ps_cm.tile([NCL_SH, 384], DT, tag=f"cm{h}",
                                   name=f"cm_{ph}_{h}") for h in range(2)]

            cm_halves = new_cm_halves("a")

            def cluster_tile(t, t_first, t_last):
                k = LAST_K if t == NT_CL - 1 else 128
                cl_t = clp.tile([128, D], DT, tag="cl")
                nc.sync.dma_start(cl_t[:k, :], cl[t * 128:t * 128 + k, :])
                cl_b = clbp.tile([128, D], BF, tag="clb", name="cl_b")
                nc.vector.tensor_copy(cl_b[:k, :], cl_t[:k, :])
                for h in range(2):
                    nc.tensor.matmul(cm_halves[h][:],
                                     map_sb[:k, t * NCL_SH:(t + 1) * NCL_SH],
                                     cl_b[:k, h * 384:(h + 1) * 384],
                                     start=(t == t_first), stop=(t == t_last))

            for t in range(4):
                cluster_tile(t, 0, NT_CL_A - 1)

            # W_h loads deferred past the first cluster tiles so the cluster
            # stream (the DMA-bound critical path) starts immediately
            wh_sb = const.tile([128, DC, NY], DT)
            wh_re = wh.rearrange("(c p) n -> p c n", p=128)
            for c in range(DC):
                nc.sync.dma_start(wh_sb[:, c, :], wh_re[:, c, :])

            # prefetch all train-embedding row tiles; compute interleaves later
            te_tiles = []
            for rr in range(NT_TE):
                te_t = tep.tile([128, D], DT, tag="te", name=f"te_t{rr}")
                nc.sync.dma_start(te_t[:], te[rr * 128:(rr + 1) * 128, :])
                te_tiles.append(te_t)

            # ---- Newton-Schulz inverse of gram ----
            # t0 = 1/||G||_F broadcast to a [64,1] per-partition scalar column
            h2 = small.tile([NCPT, NCPT], DT, tag="h2")
            nc.scalar.square(h2[:], G_sb[:])
            hrow = small.tile([NCPT, 1], DT, tag="hrow")
            nc.vector.tensor_reduce(hrow[:], h2[:], axis=mybir.AxisListType.X,
                                    op=mybir.AluOpType.add)
            hs_ps = ps_small.tile([1, 1], DT, tag="ps_small")
            nc.tensor.matmul(hs_ps[:], ones64[:], hrow[:], start=True, stop=True)
            fro = small.tile([1, 1], DT, tag="fro")
            nc.scalar.sqrt(fro[:], hs_ps[:])
            tinv = small.tile([1, 1], DT, tag="tinv")
            nc.vector.reciprocal(tinv[:], fro[:])
            tc_ps = ps_small.tile([NCPT, 1], DT, tag="ps_small")
            nc.tensor.matmul(tc_ps[:], ones1[:], tinv[:], start=True, stop=True)
            tcol = small.tile([NCPT, 1], DT, tag="tcol")
            nc.vector.tensor_copy(tcol[:], tc_ps[:])

            X = nsx.tile([NCPT, NCPT], DT, tag="X")
            nc.vector.tensor_copy(X[:], ident[:NCPT, :NCPT])
            nc.vector.tensor_scalar_mul(X[:], X[:], tcol[:])
            for i in range(NS_ITERS):
                e_ps = ps_small.tile([NCPT, NCPT], DT, tag="ps_small")
                nc.tensor.matmul(e_ps[:], G_sb[:], X[:], start=True, stop=True)
                e_sb = small.tile([NCPT, NCPT], DT, tag="esb")
                nc.scalar.copy(e_sb[:], e_ps[:])
                xe_ps = ps_small.tile([NCPT, NCPT], DT, tag="ps_small")
                nc.tensor.matmul(xe_ps[:], X[:], e_sb[:], start=True, stop=True)
                xn = nsx.tile([NCPT, NCPT], DT, tag="X")
                nc.vector.scalar_tensor_tensor(
                    out=xn[:], in0=X[:], scalar=2.0, in1=xe_ps[:],
                    op0=mybir.AluOpType.mult, op1=mybir.AluOpType.subtract)
                X = xn

            # ---- B' = [inv(gram) @ (concept.T @ W_h); b_h]  (65 x 1000) ----
            cw_sb = small.tile([NCPT, NY], DT, tag="cwsb")
            for hh in range(2):
                cw_ps = ps_wide.tile([NCPT, NYH], DT, tag="ps_wide")
                for c in range(DC):
                    nc.tensor.matmul(cw_ps[:],
                                     co_c(c), wh_sb[:, c, hh * NYH:(hh + 1) * NYH],
                                     start=(c == 0), stop=(c == DC - 1))
                nc.vector.tensor_copy(cw_sb[:, hh * NYH:(hh + 1) * NYH], cw_ps[:])
            bp_sb = const.tile([65, NY], DT)
            for hh in range(2):
                b_ps = ps_wide.tile([NCPT, NYH], DT, tag="ps_wide")
                nc.tensor.matmul(b_ps[:], X[:], cw_sb[:, hh * NYH:(hh + 1) * NYH],
                                 start=True, stop=True)
                nc.vector.tensor_copy(bp_sb[:NCPT, hh * NYH:(hh + 1) * NYH], b_ps[:])
            nc.gpsimd.dma_start(bp_sb[NCPT:NCPT + 1, :], bh[None, :])

            # ---- interleave: clusters stream + y_pred path in groups of 4
            # row tiles, so the A^T matmuls run at N=512 (fewer, wider
            # instructions: per-matmul dispatch overhead is ~200ns)
            def te_group(g):
                tet4 = tetp.tile([128, DC, 512], DT, tag="tet", name=f"tet{g}")
                for j in range(4):
                    te_t = te_tiles[4 * g + j]
                    for c in range(DC):
                        tp_ps = ps_small.tile([128, 128], DT, tag="ps_small")
                        nc.tensor.transpose(tp_ps[:], te_t[:, c * 128:(c + 1) * 128],
                                            ident[:])
                        nc.vector.tensor_copy(tet4[:, c, j * 128:(j + 1) * 128],
                                              tp_ps[:])
                at_ps = ps_wide.tile([NCPT, 512], DT, tag="ps_wide")
                for c in range(DC):
                    nc.tensor.matmul(at_ps[:], co_c(c), tet4[:, c, :],
                                     start=(c == 0), stop=(c == DC - 1))
                a_sb = apool.tile([65, 512], DT, tag="a")
                nc.vector.tensor_copy(a_sb[:NCPT, :], at_ps[:])
                nc.vector.memset(a_sb[NCPT:NCPT + 1, :], 1.0)
                for j in range(4):
                    r = 4 * g + j
                    y_sb = ypool.tile([128, NY], DT, tag="y")
                    for hh in range(2):
                        y_ps = ps_wide.tile([128, NYH], DT, tag="ps_wide")
                        nc.tensor.matmul(y_ps[:], a_sb[:, j * 128:(j + 1) * 128],
                                         bp_sb[:, hh * NYH:(hh + 1) * NYH],
                                         start=True, stop=True)
                        nc.vector.tensor_copy(y_sb[:, hh * NYH:(hh + 1) * NYH],
                                              y_ps[:])
                    nc.gpsimd.dma_start(y_out[r * 128:(r + 1) * 128, :], y_sb[:])

            # CM phase finalize: copy psum halves out, transpose to CM^T,
            # T += CM^T @ concept accumulated in SBUF t_acc across phases
            t_acc = small.tile([NCL_SH, NCPT], DT, tag="tacc")

            def cm_finalize(ph, first_phase):
                cm_sb = small.tile([NCL_SH, D], DT, tag="cmsb",
                                   name=f"cm_sb_{ph}")
                for h in range(2):
                    nc.vector.tensor_copy(cm_sb[:, h * 384:(h + 1) * 384],
                                          cm_halves[h][:])
                cmt = small.tile([128, DC, NCL_SH], DT, tag="cmt",
                                 name=f"cmt_{ph}")
                for c in range(DC):
                    tp_ps = ps_small.tile([128, 128], DT, tag="ps_small")
                    nc.tensor.transpose(tp_ps[:, :NCL_SH],
                                        cm_sb[:, c * 128:(c + 1) * 128],
                                        ident[:NCL_SH, :NCL_SH])
                    nc.vector.tensor_copy(cmt[:, c, :], tp_ps[:, :NCL_SH])
                t_ps = ps_small.tile([NCL_SH, NCPT], DT, tag="ps_small")
                for c in range(DC):
                    nc.tensor.matmul(t_ps[:], cmt[:, c, :], co_c(c),
                                     start=(c == 0), stop=(c == DC - 1))
                if first_phase:
                    nc.vector.tensor_copy(t_acc[:], t_ps[:])
                else:
                    nc.vector.tensor_add(out=t_acc[:], in0=t_acc[:], in1=t_ps[:])

            # front-load te compute (B' is ready ~20us in) so the kernel tail
            # is pure cluster streaming + the short loss finalize
            for t in range(4, NT_CL):
                if t == NT_CL_A:
                    cm_finalize("a", True)
                    cm_halves = new_cm_halves("b")
                cluster_tile(t, NT_CL_A if t >= NT_CL_A else 0,
                             (NT_CL_A - 1) if t < NT_CL_A else (NT_CL - 1))
                if t == 12:
                    te_group(0)
                if t == 30:
                    te_group(1)

            # ---- loss partials tail: second CM phase, S0, G0p, colsums ----
            cm_finalize("b", False)
            s0 = small.tile([NCL_SH, NCPT], DT, tag="s0")
            nc.scalar.activation(s0[:], t_acc[:], mybir.ActivationFunctionType.Abs)
            g0_ps = ps_small.tile([NCPT, NCPT], DT, tag="ps_small")
            nc.tensor.matmul(g0_ps[:], s0[:], s0[:], start=True, stop=True)
            cs_ps = ps_small.tile([1, NCPT], DT, tag="ps_small")
            nc.tensor.matmul(cs_ps[:], ones63[:], s0[:], start=True, stop=True)
            g0_sb = small.tile([NCPT, NCPT], DT, tag="g0sb")
            nc.vector.tensor_copy(g0_sb[:], g0_ps[:])
            cs_sb = small.tile([1, NCPT], DT, tag="cssb")
            nc.vector.tensor_copy(cs_sb[:], cs_ps[:])
            nc.gpsimd.dma_start(g0_out[:], g0_sb[:])
            nc.gpsimd.dma_start(cs_out[:], cs_sb[:])

    nc.finalize()
    return nc


def _get_program() -> bass.Bass:
    global _cached_nc
    if _cached_nc is None:
        _cached_nc = _build_program()
    return _cached_nc


def kernel(train_embedding, clusters, concept, W_h, b_h):
    global LAST_EXEC_NS, LAST_TRACE_PATH, LAST_RESULTS
    te = np.ascontiguousarray(np.asarray(train_embedding, dtype=FP32))
    cl = np.asarray(clusters, dtype=FP32)
    co = np.ascontiguousarray(np.asarray(concept, dtype=FP32))
    wh = np.ascontiguousarray(np.asarray(W_h, dtype=FP32))
    bh = np.ascontiguousarray(np.asarray(b_h, dtype=FP32))
    # [768, 64] -> [128, 6*64]: partition-contiguous layout for one clean DMA
    co_p = np.ascontiguousarray(
        co.reshape(DC, 128, NCPT).transpose(1, 0, 2).reshape(128, DC * NCPT))

    # shard: batch 8 x 1024; clusters padded to 504 = 8 x 63, each shard
    # flattened to [6300, 768] (zero rows for the 4 pad clusters contribute 0)
    cl_pad = np.zeros((N_CORES * NCL_SH, CS, D), dtype=FP32)
    cl_pad[:NCL] = cl
    in_maps = []
    for i in range(N_CORES):
        sh = cl_pad[i * NCL_SH:(i + 1) * NCL_SH].reshape(ROWS_SH, D)
        in_maps.append({
            "te": np.ascontiguousarray(te[i * BS_SH:(i + 1) * BS_SH]),
            "cl": np.ascontiguousarray(sh),
            "co_p": co_p,
            "wh": wh,
            "bh": bh,
        })

    nc = _get_program()
    res = run_bass_kernel_spmd(nc, in_maps, list(range(N_CORES)), trace=TRACE)
    LAST_EXEC_NS = res.exec_time_ns
    LAST_TRACE_PATH = None
    if res.instructions_and_trace is not None:
        LAST_TRACE_PATH = res.instructions_and_trace[1]
    LAST_RESULTS = res

    y = np.concatenate([res.results[i]["Y"] for i in range(N_CORES)], axis=0)
    g0 = np.zeros((NCPT, NCPT), dtype=np.float64)
    cs1 = np.zeros(NCPT, dtype=np.float64)
    for i in range(N_CORES):
        g0 += res.results[i]["G0p"].astype(np.float64)
        cs1 += res.results[i]["cs1"][0].astype(np.float64)

    # host finalize of the two scalar losses (sum of per-core partials)
    c = np.maximum(np.sqrt(np.diag(g0)), EPS)
    l1 = (cs1 / c).sum()
    gn = g0 / np.outer(c, c)
    l2 = gn.sum() - np.trace(gn)
    return (y, np.asarray(l1, dtype=FP32), np.asarray(l2, dtype=FP32))


# revision 52
# speedup vs baseline: 1.0604x; 1.0222x over previous
"""Trainium2 Bass kernel for the ConceptNet-style module.

Computes, distributed data-parallel over 8 NeuronCores:
    y_pred = train_embedding @ concept @ inv(concept.T@concept) @ (concept.T@W_h) + b_h
    L1     = sum(score_norm);  L2 = sum(G) - trace(G),  G = score_norm.T@score_norm
where score_norm is the column-normalized |mean_cs(clusters) @ concept_normalized|
(the concept column normalization cancels exactly inside the score normalization,
so the kernel works with |cluster_sum @ concept| directly — uniform positive
column scalings cancel in both L1 and L2).

Sharding (hardcoded): train_embedding split 8 x [1024, 768] along batch;
clusters padded 500->504 and split 8 x [63, 100, 768] along the cluster dim
(each shard flattened to [6300, 768], zero-padded to [6400, 768]); concept,
W_h, b_h replicated. Each core returns its y_pred shard plus tiny partial
reductions (G0p = S0.T@S0 [64,64] and column sums of S0 [64]); the host sums
partials across cores and finishes the two scalar losses (the "all-reduce the
scalar sparsity losses" step of the data-parallel recipe).

inv(gram) is computed on-device with Newton-Schulz iterations (X0 = I/||G||_F,
X <- 2X - X G X), which converges to fp32 accuracy in <=10 iterations for this
well-conditioned 64x64 Gram matrix.
"""

import os
import sys

import ml_dtypes
import numpy as np

for _p in ("/opt/trn_rl_repo", os.path.expanduser("~/.axon_site/_ro/trn_rl_repo")):
    if os.path.isdir(_p) and _p not in sys.path:
        sys.path.insert(0, _p)

import concourse.bass as bass
import concourse.mybir as mybir
from concourse import bacc
from concourse.bass_utils import run_bass_kernel_spmd
from concourse.masks import make_identity
from concourse.tile import TileContext

EPS = 1e-12
N_CORES = 8
BS, D, NCPT, NCL, CS, NY = 8192, 768, 64, 500, 100, 1000
BS_SH = BS // N_CORES            # 1024 batch rows per core
NCL_SH = 63                      # ceil(500/8) clusters per core (504 padded)
ROWS_SH = NCL_SH * CS            # 6300
NT_CL = 50                       # ceil(6300/128); last tile has 28 valid rows
LAST_K = ROWS_SH - (NT_CL - 1) * 128  # 28
NT_CL_A = 25                     # tiles in the first CM accumulation phase
NT_TE = BS_SH // 128             # 8
DC = D // 128                    # 6 contraction chunks of 128
NYH = NY // 2                    # 500 (psum-bank-sized y/n chunks)
NS_ITERS = 8

DT = mybir.dt.float32
BF = mybir.dt.bfloat16
FP32 = np.float32

# Toggled by test harness; kernel() stores profiling results here.
TRACE = False
LAST_EXEC_NS = None
LAST_TRACE_PATH = None
LAST_RESULTS = None

_cached_nc = None


def _cluster_map() -> np.ndarray:
    """Block-ones row->cluster map, pre-laid-out as [128, NT_CL*63] so the
    SBUF load is one DMA with contiguous per-partition rows (map[p, t*63+m]
    = 1 iff global row t*128+p belongs to cluster m).

    bf16: entries are 0/1 (exact), and the cluster-sum path is scale-invariant
    in the final losses, so low precision here only rounds the summands."""
    m = np.zeros((NT_CL * 128, NCL_SH), dtype=ml_dtypes.bfloat16)
    g = np.arange(ROWS_SH)
    m[g, g // CS] = 1.0
    return np.ascontiguousarray(
        m.reshape(NT_CL, 128, NCL_SH).transpose(1, 0, 2).reshape(128, NT_CL * NCL_SH))


def _build_program() -> bass.Bass:
    # Bacc (not raw Bass): its compile() legalizes matmul sync waits
    # (move_matmul_waits_to_ldweights + nop/event fusion) — the trn2 LDWEIGHTS
    # ISA struct carries no wait fields, so raw-Bass TileContext programs with
    # cross-engine matmul deps fail walrus codegen ("Too many sync wait").
    nc = bacc.Bacc()

    te = nc.declare_dram_parameter("te", [BS_SH, D], DT, isOutput=False)
    cl = nc.declare_dram_parameter("cl", [ROWS_SH, D], DT, isOutput=False)
    # concept pre-transposed on host to [128, DC*64] (partition-contiguous)
    co_p = nc.declare_dram_parameter("co_p", [128, DC * NCPT], DT, isOutput=False)
    wh = nc.declare_dram_parameter("wh", [D, NY], DT, isOutput=False)
    bh = nc.declare_dram_parameter("bh", [NY], DT, isOutput=False)
    y_out = nc.declare_dram_parameter("Y", [BS_SH, NY], DT, isOutput=True)
    g0_out = nc.declare_dram_parameter("G0p", [NCPT, NCPT], DT, isOutput=True)
    cs_out = nc.declare_dram_parameter("cs1", [1, NCPT], DT, isOutput=True)

    cmap = nc.inline_tensor(_cluster_map(), name="cmap")

    with TileContext(nc) as tc:
        with (
            tc.tile_pool(name="const", bufs=1) as const,
            tc.tile_pool(name="small", bufs=4) as small,
            tc.tile_pool(name="nsx", bufs=3) as nsx,
            tc.tile_pool(name="clp", bufs=14) as clp,
            tc.tile_pool(name="clbp", bufs=7) as clbp,
            tc.tile_pool(name="tep", bufs=8) as tep,
            tc.tile_pool(name="tetp", bufs=2) as tetp,
            tc.tile_pool(name="ap", bufs=2) as apool,
            tc.tile_pool(name="yp", bufs=3) as ypool,
            tc.tile_pool(name="ps_small", bufs=2, space="PSUM") as ps_small,
            tc.tile_pool(name="ps_cm", bufs=1, space="PSUM") as ps_cm,
            tc.tile_pool(name="ps_wide", bufs=2, space="PSUM") as ps_wide,
        ):
            # ---- constants / replicated operands ----
            ident = const.tile([128, 128], DT)
            make_identity(nc, ident)
            ones63 = const.tile([63, 1], DT)
            nc.vector.memset(ones63[:], 1.0)
            ones64 = const.tile([NCPT, 1], DT)
            nc.vector.memset(ones64[:], 1.0)
            ones1 = const.tile([1, NCPT], DT)
            nc.vector.memset(ones1[:], 1.0)

            co_sb = const.tile([128, DC * NCPT], DT)
            nc.sync.dma_start(co_sb[:], co_p[:])

            map_sb = const.tile([128, NT_CL * NCL_SH], BF)
            _mh = NT_CL_A * NCL_SH
            nc.sync.dma_start(map_sb[:, :_mh], cmap[:, :_mh])
            nc.sync.dma_start(map_sb[:, _mh:], cmap[:, _mh:])

            def co_c(c):
                return co_sb[:, c * NCPT:(c + 1) * NCPT]

            # ---- gram = concept.T @ concept ----
            gram_ps = ps_small.tile([NCPT, NCPT], DT, tag="ps_small")
            for c in range(DC):
                nc.tensor.matmul(gram_ps[:], co_c(c), co_c(c),
                                 start=(c == 0), stop=(c == DC - 1))
            G_sb = small.tile([NCPT, NCPT], DT, tag="gsb")
            nc.vector.tensor_copy(G_sb[:], gram_ps[:])

            # ---- clusters: start streaming early; CM accumulates in PSUM ----
            # one PSUM tile per 384-wide half: a matmul output must stay
            # within a single 512-f32 PSUM bank. Two accumulation phases
            # (tiles 0..24, 25..49) so most of the transpose/T finalize chain
            # hides mid-kernel instead of trailing the last cluster DMA.
            def new_cm_halves(ph):
                return [ps_cm.tile([NCL_SH, 384], DT, tag=f"cm{h}",
                                   name=f"cm_{ph}_{h}") for h in range(2)]

            cm_halves = new_cm_halves("a")

            def cluster_tile(t, t_first, t_last):
                k = LAST_K if t == NT_CL - 1 else 128
                cl_t = clp.tile([128, D], DT, tag="cl")
                nc.sync.dma_start(cl_t[:k, :], cl[t * 128:t * 128 + k, :])
                cl_b = clbp.tile([128, D], BF, tag="clb", name="cl_b")
                nc.vector.tensor_copy(cl_b[:k, :], cl_t[:k, :])
                for h in range(2):
                    nc.tensor.matmul(cm_halves[h][:],
                                     map_sb[:k, t * NCL_SH:(t + 1) * NCL_SH],
                                     cl_b[:k, h * 384:(h + 1) * 384],
                                     start=(t == t_first), stop=(t == t_last))

            for t in range(4):
                cluster_tile(t, 0, NT_CL_A - 1)

            # W_h loads deferred past the first cluster tiles so the cluster
            # stream (the DMA-bound critical path) starts immediately
            wh_sb = const.tile([128, DC, NY], DT)
            wh_re = wh.rearrange("(c p) n -> p c n", p=128)
            for c in range(DC):
                nc.sync.dma_start(wh_sb[:, c, :], wh_re[:, c, :])

            # prefetch all train-embedding row tiles; compute interleaves later
            te_tiles = []
            for rr in range(NT_TE):
                te_t = tep.tile([128, D], DT, tag="te", name=f"te_t{rr}")
                nc.sync.dma_start(te_t[:], te[rr * 128:(rr + 1) * 128, :])
                te_tiles.append(te_t)

            # ---- Newton-Schulz inverse of gram ----
            # t0 = 1/||G||_F broadcast to a [64,1] per-partition scalar column
            h2 = small.tile([NCPT, NCPT], DT, tag="h2")
            nc.scalar.square(h2[:], G_sb[:])
            hrow = small.tile([NCPT, 1], DT, tag="hrow")
            nc.vector.tensor_reduce(hrow[:], h2[:], axis=mybir.AxisListType.X,
                                    op=mybir.AluOpType.add)
            hs_ps = ps_small.tile([1, 1], DT, tag="ps_small")
            nc.tensor.matmul(hs_ps[:], ones64[:], hrow[:], start=True, stop=True)
            fro = small.tile([1, 1], DT, tag="fro")
            nc.scalar.sqrt(fro[:], hs_ps[:])
            tinv = small.tile([1, 1], DT, tag="tinv")
            nc.vector.reciprocal(tinv[:], fro[:])
            tc_ps = ps_small.tile([NCPT, 1], DT, tag="ps_small")
            nc.tensor.matmul(tc_ps[:], ones1[:], tinv[:], start=True, stop=True)
            tcol = small.tile([NCPT, 1], DT, tag="tcol")
            nc.vector.tensor_copy(tcol[:], tc_ps[:])

            X = nsx.tile([NCPT, NCPT], DT, tag="X")
            nc.vector.tensor_copy(X[:], ident[:NCPT, :NCPT])
            nc.vector.tensor_scalar_mul(X[:], X[:], tcol[:])
            for i in range(NS_ITERS):
                e_ps = ps_small.tile([NCPT, NCPT], DT, tag="ps_small")
                nc.tensor.matmul(e_ps[:], G_sb[:], X[:], start=True, stop=True)
                e_sb = small.tile([NCPT, NCPT], DT, tag="esb")
                nc.scalar.copy(e_sb[:], e_ps[:])
                xe_ps = ps_small.tile([NCPT, NCPT], DT, tag="ps_small")
                nc.tensor.matmul(xe_ps[:], X[:], e_sb[:], start=True, stop=True)
                xn = nsx.tile([NCPT, NCPT], DT, tag="X")
                nc.vector.scalar_tensor_tensor(
                    out=xn[:], in0=X[:], scalar=2.0, in1=xe_ps[:],
                    op0=mybir.AluOpType.mult, op1=mybir.AluOpType.subtract)
                X = xn

            # ---- B' = [inv(gram) @ (concept.T @ W_h); b_h]  (65 x 1000) ----
            cw_sb = small.tile([NCPT, NY], DT, tag="cwsb")
            for hh in range(2):
                cw_ps = ps_wide.tile([NCPT, NYH], DT, tag="ps_wide")
                for c in range(DC):
                    nc.tensor.matmul(cw_ps[:],
                                     co_c(c), wh_sb[:, c, hh * NYH:(hh + 1) * NYH],
                                     start=(c == 0), stop=(c == DC - 1))
                nc.vector.tensor_copy(cw_sb[:, hh * NYH:(hh + 1) * NYH], cw_ps[:])
            bp_sb = const.tile([65, NY], DT)
            for hh in range(2):
                b_ps = ps_wide.tile([NCPT, NYH], DT, tag="ps_wide")
                nc.tensor.matmul(b_ps[:], X[:], cw_sb[:, hh * NYH:(hh + 1) * NYH],
                                 start=True, stop=True)
                nc.vector.tensor_copy(bp_sb[:NCPT, hh * NYH:(hh + 1) * NYH], b_ps[:])
            nc.gpsimd.dma_start(bp_sb[NCPT:NCPT + 1, :], bh[None, :])

            # ---- interleave: clusters stream + y_pred path in groups of 4
            # row tiles, so the A^T matmuls run at N=512 (fewer, wider
            # instructions: per-matmul dispatch overhead is ~200ns)
            def te_group(g):
                tet4 = tetp.tile([128, DC, 512], DT, tag="tet", name=f"tet{g}")
                for j in range(4):
                    te_t = te_tiles[4 * g + j]
                    for c in range(DC):
                        tp_ps = ps_small.tile([128, 128], DT, tag="ps_small")
                        nc.tensor.transpose(tp_ps[:], te_t[:, c * 128:(c + 1) * 128],
                                            ident[:])
                        nc.vector.tensor_copy(tet4[:, c, j * 128:(j + 1) * 128],
                                              tp_ps[:])
                at_ps = ps_wide.tile([NCPT, 512], DT, tag="ps_wide")
                for c in range(DC):
                    nc.tensor.matmul(at_ps[:], co_c(c), tet4[:, c, :],
                                     start=(c == 0), stop=(c == DC - 1))
                a_sb = apool.tile([65, 512], DT, tag="a")
                nc.vector.tensor_copy(a_sb[:NCPT, :], at_ps[:])
                nc.vector.memset(a_sb[NCPT:NCPT + 1, :], 1.0)
                for j in range(4):
                    r = 4 * g + j
                    y_sb = ypool.tile([128, NY], DT, tag="y")
                    for hh in range(2):
                        y_ps = ps_wide.tile([128, NYH], DT, tag="ps_wide")
                        nc.tensor.matmul(y_ps[:], a_sb[:, j * 128:(j + 1) * 128],
                                         bp_sb[:, hh * NYH:(hh + 1) * NYH],
                                         start=True, stop=True)
                        nc.vector.tensor_copy(y_sb[:, hh * NYH:(hh + 1) * NYH],
                                              y_ps[:])
                    nc.gpsimd.dma_start(y_out[r * 128:(r + 1) * 128, :], y_sb[:])

            # CM phase finalize: copy psum halves out, transpose to CM^T,
            # T += CM^T @ concept accumulated in SBUF t_acc across phases
            t_acc = small.tile([NCL_SH, NCPT], DT, tag="tacc")

            def cm_finalize(ph, first_phase):
                cm_sb = small.tile([NCL_SH, D], DT, tag="cmsb",
                                   name=f"cm_sb_{ph}")
                for h in range(2):
                    nc.vector.tensor_copy(cm_sb[:, h * 384:(h + 1) * 384],
                                          cm_halves[h][:])
                cmt = small.tile([128, DC, NCL_SH], DT, tag="cmt",
                                 name=f"cmt_{ph}")
                for c in range(DC):
                    tp_ps = ps_small.tile([128, 128], DT, tag="ps_small")
                    nc.tensor.transpose(tp_ps[:, :NCL_SH],
                                        cm_sb[:, c * 128:(c + 1) * 128],
                                        ident[:NCL_SH, :NCL_SH])
                    nc.vector.tensor_copy(cmt[:, c, :], tp_ps[:, :NCL_SH])
                t_ps = ps_small.tile([NCL_SH, NCPT], DT, tag="ps_small")
                for c in range(DC):
                    nc.tensor.matmul(t_ps[:], cmt[:, c, :], co_c(c),
                                     start=(c == 0), stop=(c == DC - 1))
                if first_phase:
                    nc.vector.tensor_copy(t_acc[:], t_ps[:])
                else:
                    nc.vector.tensor_add(out=t_acc[:], in0=t_acc[:], in1=t_ps[:])

            # front-load te compute (B' is ready ~20us in) so the kernel tail
            # is pure cluster streaming + the short loss finalize
            for t in range(4, NT_CL):
                if t == NT_CL_A:
                    cm_finalize("a", True)
                    cm_halves = new_cm_halves("b")
                cluster_tile(t, NT_CL_A if t >= NT_CL_A else 0,
                             (NT_CL_A - 1) if t < NT_CL_A else (NT_CL - 1))
                if t == 10:
                    te_group(0)
                if t == 26:
                    te_group(1)

            # ---- loss partials tail: second CM phase, S0, G0p, colsums ----
            cm_finalize("b", False)
            s0 = small.tile([NCL_SH, NCPT], DT, tag="s0")
            nc.scalar.activation(s0[:], t_acc[:], mybir.ActivationFunctionType.Abs)
            g0_ps = ps_small.tile([NCPT, NCPT], DT, tag="ps_small")
            nc.tensor.matmul(g0_ps[:], s0[:], s0[:], start=True, stop=True)
            cs_ps = ps_small.tile([1, NCPT], DT, tag="ps_small")
            nc.tensor.matmul(cs_ps[:], ones63[:], s0[:], start=True, stop=True)
            g0_sb = small.tile([NCPT, NCPT], DT, tag="g0sb")
            nc.vector.tensor_copy(g0_sb[:], g0_ps[:])
            cs_sb = small.tile([1, NCPT], DT, tag="cssb")
            nc.vector.tensor_copy(cs_sb[:], cs_ps[:])
            nc.gpsimd.dma_start(g0_out[:], g0_sb[:])
            nc.gpsimd.dma_start(cs_out[:], cs_sb[:])

    nc.finalize()
    return nc


def _get_program() -> bass.Bass:
    global _cached_nc
    if _cached_nc is None:
        _cached_nc = _build_program()
    return _cached_nc


def kernel(train_embedding, clusters, concept, W_h, b_h):
    global LAST_EXEC_NS, LAST_TRACE_PATH, LAST_RESULTS
    te = np.ascontiguousarray(np.asarray(train_embedding, dtype=FP32))
    cl = np.asarray(clusters, dtype=FP32)
    co = np.ascontiguousarray(np.asarray(concept, dtype=FP32))
    wh = np.ascontiguousarray(np.asarray(W_h, dtype=FP32))
    bh = np.ascontiguousarray(np.asarray(b_h, dtype=FP32))
    # [768, 64] -> [128, 6*64]: partition-contiguous layout for one clean DMA
    co_p = np.ascontiguousarray(
        co.reshape(DC, 128, NCPT).transpose(1, 0, 2).reshape(128, DC * NCPT))

    # shard: batch 8 x 1024; clusters padded to 504 = 8 x 63, each shard
    # flattened to [6300, 768] (zero rows for the 4 pad clusters contribute 0)
    cl_pad = np.zeros((N_CORES * NCL_SH, CS, D), dtype=FP32)
    cl_pad[:NCL] = cl
    in_maps = []
    for i in range(N_CORES):
        sh = cl_pad[i * NCL_SH:(i + 1) * NCL_SH].reshape(ROWS_SH, D)
        in_maps.append({
            "te": np.ascontiguousarray(te[i * BS_SH:(i + 1) * BS_SH]),
            "cl": np.ascontiguousarray(sh),
            "co_p": co_p,
            "wh": wh,
            "bh": bh,
        })

    nc = _get_program()
    res = run_bass_kernel_spmd(nc, in_maps, list(range(N_CORES)), trace=TRACE)
    LAST_EXEC_NS = res.exec_time_ns
    LAST_TRACE_PATH = None
    if res.instructions_and_trace is not None:
        LAST_TRACE_PATH = res.instructions_and_trace[1]
    LAST_RESULTS = res

    y = np.concatenate([res.results[i]["Y"] for i in range(N_CORES)], axis=0)
    g0 = np.zeros((NCPT, NCPT), dtype=np.float64)
    cs1 = np.zeros(NCPT, dtype=np.float64)
    for i in range(N_CORES):
        g0 += res.results[i]["G0p"].astype(np.float64)
        cs1 += res.results[i]["cs1"][0].astype(np.float64)

    # host finalize of the two scalar losses (sum of per-core partials)
    c = np.maximum(np.sqrt(np.diag(g0)), EPS)
    l1 = (cs1 / c).sum()
    gn = g0 / np.outer(c, c)
    l2 = gn.sum() - np.trace(gn)
    return (y, np.asarray(l1, dtype=FP32), np.asarray(l2, dtype=FP32))
